# revision 6
# baseline (speedup 1.0000x reference)
"""Causal self-attention (S=2048, D=1024, H=16) on 8 Trainium2 NeuronCores.

Sharding: tensor-parallel over heads; core c owns heads 2c, 2c+1.

v2 pipeline (fp8-heavy):
  - Weights are pre-scaled x32 on the host so both hidden_states and weights
    sit in fp8e4's normal range; W_proj is divided by 32 to compensate.
  - hidden_states are shipped as fp8 (h8) plus an fp8 residual (r8 = hs - h8).
  - Q,K projections: fp8 DoubleRow matmuls (2 contraction rows/cycle).
    V projection: 3-term fp8 split (h8.w8 + h8.u8 + r8.w8, u8 = w residual)
    giving bf16-level accuracy at half the cycles.
  - Q,K psum -> fp8 copies -> single-descriptor DMA repack into the
    [32, 2, S] DoubleRow layout (slot pairing d = 2p+i).
  - Logits: fp8 DoubleRow (256 cyc per 128x512 block), exp on ScalarE (bf16),
    causal mask multiply on DVE, PV in bf16 with a ones-column denominator.
  - Softmax denominators: DVE reciprocal (no Ln/Exp act-table thrash),
    broadcast via K=1 matmul, fused into the projection input.
  - Projection per 128-row chunk right after its query-chunk is normalized;
    output written as bf16 (host sums partials in f32 and adds b_proj).
  - Emission order = tensor-engine schedule: projection/next-chunk-QKV work is
    interleaved between attention groups as PE filler during ScalarE waits.
"""

import math
from collections import deque
from contextlib import ExitStack

import numpy as np

import concourse.bacc as bacc
import concourse.mybir as mybir
import concourse.tile as tile
from concourse.bass_utils import run_bass_kernel_spmd

S, D, H = 2048, 1024, 16
HS = D // H  # 64 head size
P = 128
NCORES = 8
HPC = H // NCORES  # 2 heads per core
CD = HPC * HS  # 128 per-core head dims
KO = D // P  # 8 contraction tiles
KO2 = KO // 2  # 4 o-pairs for DoubleRow
NQC = S // 512  # 4 query chunks
NSC = S // P  # 16 sequence chunks of 128
WSCALE = 32.0
SCALE = 1.0 / (math.sqrt(S) * WSCALE * WSCALE)

F32 = mybir.dt.float32
F32R = mybir.dt.float32r
BF16 = mybir.dt.bfloat16
FP8 = mybir.dt.float8e4
DR = mybir.MatmulPerfMode.DoubleRow

import ml_dtypes

NP_BF16 = ml_dtypes.bfloat16
NP_FP8 = ml_dtypes.float8_e4m3


def _build():
    nc = bacc.Bacc(
        "TRN2", target_bir_lowering=False, debug=False, num_devices=NCORES
    )

    h8d = nc.dram_tensor("h8d", [P, KO, S], FP8, kind="ExternalInput")
    r8d = nc.dram_tensor("r8d", [P, KO, S], FP8, kind="ExternalInput")
    w8d = nc.dram_tensor("w8d", [P, KO2, 2, 3 * P], FP8, kind="ExternalInput")
    u8d = nc.dram_tensor("u8d", [P, KO2, 2, P], FP8, kind="ExternalInput")
    b_qkv = nc.dram_tensor("b_qkv", [P, 3], F32, kind="ExternalInput")
    w_p = nc.dram_tensor("w_p", [CD, D], F32R, kind="ExternalInput")
    msk = nc.dram_tensor("msk", [P, 896], BF16, kind="ExternalInput")
    iden_b = nc.dram_tensor("iden_b", [P, P], BF16, kind="ExternalInput")
    vones = nc.dram_tensor("vones", [P, NSC], BF16, kind="ExternalInput")
    ones_r = nc.dram_tensor("ones_r", [1, P], F32R, kind="ExternalInput")
    out = nc.dram_tensor("out", [S, D], BF16, kind="ExternalOutput")

    with (
        tile.TileContext(nc) as tc,
        ExitStack() as ctx,
        nc.allow_low_precision(reason="fp8/bf16 matmul pipeline"),
    ):
        const = ctx.enter_context(tc.tile_pool(name="const", bufs=1))
        work = ctx.enter_context(tc.tile_pool(name="work", bufs=2))
        pp = ctx.enter_context(tc.tile_pool(name="pp", bufs=1, space="PSUM"))

        def psA(name):  # shared 2-bank ring
            return pp.tile([P, 2, 512], F32, tag="A", bufs=3, name=name)

        # ---- critical loads: weights + first hidden chunk
        identb = const.tile([P, P], BF16, tag="identb", name="identb")
        nc.gpsimd.dma_start(out=identb, in_=iden_b.ap())
        onesr_sb = const.tile([1, P], F32R, tag="onesr", name="onesr_sb")
        nc.gpsimd.dma_start(out=onesr_sb, in_=ones_r.ap())
        bqkv_sb = const.tile([P, 3], F32, tag="bqkv", name="bqkv_sb")
        nc.gpsimd.dma_start(out=bqkv_sb, in_=b_qkv.ap())

        w8_sb = const.tile([P, KO2, 2, 3 * P], FP8, tag="w8", name="w8_sb")
        nc.sync.dma_start(out=w8_sb, in_=w8d.ap())
        u8_sb = const.tile([P, KO2, 2, P], FP8, tag="u8", name="u8_sb")
        nc.gpsimd.dma_start(out=u8_sb, in_=u8d.ap())

        h8_sb = const.tile([P, KO, S], FP8, tag="h8", name="h8_sb")
        r8_sb = const.tile([P, KO, S], FP8, tag="r8", name="r8_sb")

        def load_chunk(n):
            nc.sync.dma_start(
                out=h8_sb[:, :, n * 512 : (n + 1) * 512],
                in_=h8d.ap()[:, :, n * 512 : (n + 1) * 512],
            )
            nc.gpsimd.dma_start(
                out=r8_sb[:, :, n * 512 : (n + 1) * 512],
                in_=r8d.ap()[:, :, n * 512 : (n + 1) * 512],
            )

        load_chunk(0)
        msk_sb = const.tile([P, 896], BF16, tag="msk", name="msk_sb")
        nc.gpsimd.dma_start(out=msk_sb, in_=msk.ap())
        load_chunk(1)

        wp_sb = const.tile([P, D], F32R, tag="wp", name="wp_sb")
        v_sb = []
        for h in range(HPC):
            vt = const.tile([P, NSC, HS + 1], BF16, tag=f"v{h}", name=f"v{h}_sb")
            nc.gpsimd.dma_start(out=vt[:, :, HS], in_=vones.ap())
            v_sb.append(vt)

        q8nat = const.tile([P, S], FP8, tag="q8nat", name="q8nat")
        k8nat = const.tile([P, S], FP8, tag="k8nat", name="k8nat")
        q8_sb = const.tile([HS, 2, S], FP8, tag="q8", name="q8_sb")
        k8_sb = const.tile([HS, 2, S], FP8, tag="k8", name="k8_sb")
        vT_sb = const.tile([P, S], BF16, tag="vT", name="vT_sb")
        u2_sb = [
            const.tile([P, 512], F32R, tag=f"u2_{qc}", name=f"u2_{qc}")
            for qc in range(NQC)
        ]
        u2n_sb = [
            const.tile([P, 512], F32R, tag=f"u2n_{qc}", name=f"u2n_{qc}")
            for qc in range(NQC)
        ]
        den_sb = {
            (qc, h): const.tile([1, 512], F32, tag=f"den_{qc}_{h}", name=f"den_{qc}_{h}")
            for qc in range(NQC)
            for h in range(HPC)
        }

        # ---- PE warm-up paced by the first chunk loads
        ps_w = psA("ps_w")
        for o2 in range(KO2):
            for rep in range(10):
                nc.tensor.matmul(
                    ps_w[:, 0, :],
                    lhsT=w8_sb[:, o2, :, 0:P],
                    rhs=h8_sb[:, 2 * o2 : 2 * o2 + 2, 0:512],
                    start=True,
                    stop=True,
                    perf_mode=DR,
                )

        # ---- phase 1: q,k (fp8 DoubleRow) + v (3-term fp8) per 512-chunk
        def emit_p1_qk(n):
            cols = slice(n * 512, (n + 1) * 512)
            for m in range(2):
                ps_m = psA("ps_qk")[:, 0, :]
                for o2 in range(KO2):
                    nc.tensor.matmul(
                        ps_m,
                        lhsT=w8_sb[:, o2, :, m * P : (m + 1) * P],
                        rhs=h8_sb[:, 2 * o2 : 2 * o2 + 2, cols],
                        start=(o2 == 0),
                        stop=(o2 == KO2 - 1),
                        perf_mode=DR,
                    )
                nat = q8nat if m == 0 else k8nat
                nc.vector.tensor_scalar_add(
                    out=nat[:, cols], in0=ps_m, scalar1=bqkv_sb[:, m : m + 1]
                )
                dst = q8_sb if m == 0 else k8_sb
                # single-descriptor fold [128,512] -> [64,2,512]: d = 2p+i
                nc.sync.dma_start(out=dst[:, :, cols], in_=nat[:, cols])

        def emit_p1_v(n):
            cols = slice(n * 512, (n + 1) * 512)
            ps_v = psA("ps_v")[:, 0, :]
            for o2 in range(KO2):
                nc.tensor.matmul(
                    ps_v,
                    lhsT=w8_sb[:, o2, :, 2 * P : 3 * P],
                    rhs=h8_sb[:, 2 * o2 : 2 * o2 + 2, cols],
                    start=(o2 == 0),
                    stop=False,
                    perf_mode=DR,
                )
            for o2 in range(KO2):
                nc.tensor.matmul(
                    ps_v,
                    lhsT=u8_sb[:, o2, :, :],
                    rhs=h8_sb[:, 2 * o2 : 2 * o2 + 2, cols],
                    start=False,
                    stop=False,
                    perf_mode=DR,
                )
            for o2 in range(KO2):
                nc.tensor.matmul(
                    ps_v,
                    lhsT=w8_sb[:, o2, :, 2 * P : 3 * P],
                    rhs=r8_sb[:, 2 * o2 : 2 * o2 + 2, cols],
                    start=False,
                    stop=(o2 == KO2 - 1),
                    perf_mode=DR,
                )
            nc.vector.tensor_scalar_add(
                out=vT_sb[:, cols], in0=ps_v, scalar1=bqkv_sb[:, 2:3]
            )

        def emit_p1_t(sc):
            # transpose one 128-key block of vT into per-head natural v
            ps_t = pp.tile([P, P], BF16, tag="A", bufs=3, name="ps_t")
            nc.tensor.transpose(ps_t, vT_sb[:, sc * P : (sc + 1) * P], identb)
            for h in range(HPC):
                nc.vector.tensor_copy(
                    out=v_sb[h][:, sc, 0:HS], in_=ps_t[:, h * HS : (h + 1) * HS]
                )

        # prologue: chunk 0 fully, so qc0 attention can start
        emit_p1_qk(0)
        emit_p1_v(0)
        for sc in range(4):
            emit_p1_t(sc)

        # ---- norm: DVE reciprocal + K=1 broadcast + one multiply per head
        def emit_norm(qc):
            rb_ps = psA("ps_rb")
            for h in range(HPC):
                rrow = work.tile([1, 512], F32R, tag=f"rr{h}", bufs=2, name="rrow")
                nc.vector.reciprocal(out=rrow, in_=den_sb[(qc, h)])
                nc.tensor.matmul(
                    rb_ps[0:HS, h, :],
                    lhsT=onesr_sb[:, 0:HS],
                    rhs=rrow,
                    start=True,
                    stop=True,
                )
            for h in range(HPC):
                nc.vector.tensor_mul(
                    out=u2n_sb[qc][h * HS : (h + 1) * HS, :],
                    in0=u2_sb[qc][h * HS : (h + 1) * HS, :],
                    in1=rb_ps[0:HS, h, :],
                )

        # ---- phase 3: projection chunk + paired bf16 output DMA
        out_t = {}

        def emit_p3(sc):
            qc = sc // 4
            f = sc % 4
            slot = psA("ps_p3")
            for dc in range(2):
                nc.tensor.matmul(
                    slot[:, dc, :],
                    lhsT=u2n_sb[qc][:, f * P : (f + 1) * P],
                    rhs=wp_sb[:, dc * 512 : (dc + 1) * 512],
                    start=True,
                    stop=True,
                )
            pair = sc // 2
            if sc % 2 == 0:
                out_t[pair] = work.tile(
                    [P, 2, 2, 512], BF16, tag="out", bufs=2, name="out_t"
                )
            nc.vector.tensor_copy(out=out_t[pair][:, sc % 2], in_=slot)
            if sc % 2 == 1:
                eng = nc.sync if pair % 2 == 0 else nc.gpsimd
                # dram rows r = 128*j + p  ->  dims (p, j, c) to match src order
                dst = out.ap()[(pair * 2) * P : (pair * 2 + 2) * P, :].rearrange(
                    "(j p) c -> p j c", j=2
                )
                eng.dma_start(
                    out=dst,
                    in_=out_t[pair].rearrange("p a b c -> p a (b c)"),
                )

        # ---- phase 2: causal attention, software-pipelined, with PE fillers
        # each filler is (deadline_qc, fn): fn must be emitted before the
        # groups of qc == deadline_qc start (Tile orders by emission)
        fillers = deque()

        def drain(k):
            for _ in range(min(k, len(fillers))):
                fillers.popleft()[1]()

        def drain_due(qc):
            rest = deque()
            while fillers:
                d, fn = fillers.popleft()
                if d <= qc:
                    fn()
                else:
                    rest.append((d, fn))
            fillers.extend(rest)

        for qc in range(NQC):
            # next chunk's phase-1 work interleaves into this qc's groups
            if qc + 1 < NQC:
                n = qc + 1
                fillers.append((n, lambda n=n: emit_p1_qk(n)))
                fillers.append((n, lambda n=n: emit_p1_v(n)))
                for sc in range(4 * n, 4 * n + 4):
                    fillers.append((n, lambda sc=sc: emit_p1_t(sc)))
                if n + 1 < NQC:
                    fillers.append((n + 1, lambda n=n: load_chunk(n + 1)))
                if n == 1:
                    fillers.append(
                        (NQC, lambda: nc.sync.dma_start(out=wp_sb, in_=w_p.ap()))
                    )
            drain_due(qc)

            ps_o = [
                pp.tile([P, 512], F32, tag="O", bufs=2, name=f"ps_o{h}")
                for h in range(HPC)
            ]
            nkb = 4 * (qc + 1)
            ngrp = nkb // 2

            def emit_pv(pend, nkb=nkb, ps_o=ps_o):
                pes, kbs, f0 = pend
                for h in range(HPC):
                    for j, kb in enumerate(kbs):
                        nc.tensor.matmul(
                            ps_o[h][0 : HS + 1, f0:512],
                            lhsT=v_sb[h][:, kb, :],
                            rhs=pes[h][:, j, f0:512],
                            start=(kb == 0),
                            stop=(kb == nkb - 1),
                        )

            pending = None
            for g in range(ngrp):
                kbs = [2 * g, 2 * g + 1]
                f0 = 256 if g == ngrp - 1 else 0
                ps_att = [psA(f"ps_att{h}") for h in range(HPC)]
                for j, kb in enumerate(kbs):
                    for h in range(HPC):
                        nc.tensor.matmul(
                            ps_att[h][:, j, f0:512],
                            lhsT=k8_sb[
                                32 * h : 32 * h + 32, :, kb * P : (kb + 1) * P
                            ],
                            rhs=q8_sb[
                                32 * h : 32 * h + 32,
                                :,
                                qc * 512 + f0 : (qc + 1) * 512,
                            ],
                            start=True,
                            stop=True,
                            perf_mode=DR,
                        )
                if pending is not None:
                    emit_pv(pending)
                drain(1)
                pes = []
                for h in range(HPC):
                    p_exp = work.tile(
                        [P, 2, 512], BF16, tag=f"pe{h}", bufs=4, name="p_exp"
                    )
                    nc.scalar.activation(
                        out=p_exp[:, :, f0:512],
                        in_=ps_att[h][:, :, f0:512],
                        func=mybir.ActivationFunctionType.Exp,
                        scale=SCALE,
                    )
                    for j, kb in enumerate(kbs):
                        jj = kb - 4 * qc
                        if jj >= 0:
                            off = 384 - 128 * jj
                            nc.vector.tensor_mul(
                                out=p_exp[:, j, f0:512],
                                in0=p_exp[:, j, f0:512],
                                in1=msk_sb[:, off + f0 : off + 512],
                            )
                    pes.append(p_exp)
                pending = (pes, kbs, f0)
            emit_pv(pending)

            for h in range(HPC):
                nc.vector.tensor_copy(
                    out=u2_sb[qc][h * HS : (h + 1) * HS, :], in_=ps_o[h][0:HS, :]
                )
                nc.vector.tensor_copy(
                    out=den_sb[(qc, h)], in_=ps_o[h][HS : HS + 1, :]
                )
            emit_norm(qc)
            for sc in range(4 * qc, 4 * qc + 4):
                fillers.append((NQC, lambda sc=sc: emit_p3(sc)))

        drain(len(fillers))

    nc.compile()
    return nc


_NC = None


def _get_nc():
    global _NC
    if _NC is None:
        _NC = _build()
    return _NC


def prepare_inputs(hidden_states, W_attn, b_attn, W_proj, b_proj):
    hs = np.asarray(hidden_states, dtype=np.float32)
    Wa = np.asarray(W_attn, dtype=np.float32) * WSCALE
    ba = np.asarray(b_attn, dtype=np.float32) * WSCALE
    Wp = np.asarray(W_proj, dtype=np.float32) / WSCALE

    hsT = np.ascontiguousarray(hs.T)  # [D, S] f32
    h8 = hsT.astype(NP_FP8)
    r8 = (hsT - h8.astype(np.float32)).astype(NP_FP8)
    # [D, S] -> [128, KO, S]
    h8d = np.ascontiguousarray(h8.reshape(KO, P, S).transpose(1, 0, 2))
    r8d = np.ascontiguousarray(r8.reshape(KO, P, S).transpose(1, 0, 2))

    pcol = np.arange(P)[:, None]
    ccol = np.arange(896)[None, :]
    msk = (pcol <= ccol - 384).astype(NP_BF16)

    in_maps = []
    for c in range(NCORES):
        q0 = c * CD
        wq = Wa[:, q0 : q0 + CD]
        wk = Wa[:, D + q0 : D + q0 + CD]
        wv = Wa[:, 2 * D + q0 : 2 * D + q0 + CD]
        wcat = np.concatenate([wq, wk, wv], axis=1)  # [D, 384]
        w8 = wcat.astype(NP_FP8)
        u8 = (wv - wv.astype(NP_FP8).astype(np.float32)).astype(NP_FP8)
        # [D, cols] -> [128, KO2, 2, cols]
        w8d = np.ascontiguousarray(
            w8.reshape(KO2, 2, P, 3 * P).transpose(2, 0, 1, 3)
        )
        u8d = np.ascontiguousarray(
            u8.reshape(KO2, 2, P, P).transpose(2, 0, 1, 3)
        )
        bq = ba[q0 : q0 + CD]
        bk = ba[D + q0 : D + q0 + CD]
        bv = ba[2 * D + q0 : 2 * D + q0 + CD]
        in_maps.append(
            {
                "h8d": h8d,
                "r8d": r8d,
                "w8d": w8d,
                "u8d": u8d,
                "b_qkv": np.ascontiguousarray(
                    np.stack([bq, bk, bv], axis=1)
                ).astype(np.float32),
                "w_p": np.ascontiguousarray(Wp[q0 : q0 + CD, :], dtype=np.float32),
                "msk": msk,
                "iden_b": np.eye(P).astype(NP_BF16),
                "ones_r": np.ones((1, P), dtype=np.float32),
                "vones": np.ones((P, NSC)).astype(NP_BF16),
            }
        )
    return in_maps


def run(inputs, trace=False):
    """Build+run the sharded kernel. Returns (full_output, BassKernelResults)."""
    in_maps = prepare_inputs(**inputs)
    nc = _get_nc()
    res = run_bass_kernel_spmd(
        nc, in_maps, core_ids=list(range(NCORES)), trace=trace
    )
    acc = np.zeros((S, D), dtype=np.float32)
    for c in range(NCORES):
        acc += res.results[c]["out"].astype(np.float32)
    acc += np.asarray(inputs["b_proj"], dtype=np.float32)
    return acc, res


def kernel(**inputs):
    out, _ = run(inputs, trace=False)
    return out


# revision 14
# speedup vs baseline: 1.1955x; 1.1955x over previous
"""Causal self-attention (S=2048, D=1024, H=16) on 8 Trainium2 NeuronCores.

Sharding: tensor-parallel over heads; core c owns heads 2c, 2c+1.

v2.1 pipeline (bf16 matmuls, restructured schedule):
  - Inputs stream as 4 chunk-contiguous 1MB descriptors (first chunk split in
    half so QKV work starts ~2us in); weights in one descriptor each.
  - Phase 1 (QKV projections into contraction layout + PE transposes of V into
    natural layout) is emitted as "filler" work interleaved between attention
    groups of the previous query chunk, keeping the in-order PE queue dense
    while ScalarE runs exp.
  - Causal masking at 128x128 sub-block granularity: only the exact diagonal
    sub-block gets a DVE mask multiply; fully-hidden regions are skipped by
    column-slicing the PV matmuls.
  - Softmax denominators: PE-transpose den rows into columns, one wide DVE
    reciprocal on [128, 4, 2] (avoids the pathological single-lane [1,512]
    reciprocal and the Ln/Exp act-table thrash), PE-transpose back, broadcast
    via K=1 matmul. Norm for chunk qc is emitted after the first group of
    qc+1 so the PE never stalls on it.
  - Projection chunks run right after their chunk's norm; output is written as
    bf16 in paired 256-row descriptors (host sums partials in f32 + b_proj).
"""

import math
from collections import deque
from contextlib import ExitStack

import numpy as np

import concourse.bacc as bacc
import concourse.mybir as mybir
import concourse.tile as tile
from concourse.bass_utils import run_bass_kernel_spmd

S, D, H = 2048, 1024, 16
HS = D // H  # 64 head size
P = 128
NCORES = 8
HPC = H // NCORES  # 2 heads per core
CD = HPC * HS  # 128 per-core head dims
KO = D // P  # 8 contraction tiles
NQC = S // 512  # 4 query chunks
NSC = S // P  # 16 sequence chunks of 128
SCALE = 1.0 / math.sqrt(S)

F32 = mybir.dt.float32
F32R = mybir.dt.float32r
BF16 = mybir.dt.bfloat16

import ml_dtypes

NP_BF16 = ml_dtypes.bfloat16


def _build():
    nc = bacc.Bacc(
        "TRN2", target_bir_lowering=False, debug=False, num_devices=NCORES
    )

    hsTd = nc.dram_tensor("hsTd", [P, NQC, KO, 512], BF16, kind="ExternalInput")
    wqd = nc.dram_tensor("wqd", [P, KO, 3 * P], BF16, kind="ExternalInput")
    b_qkv = nc.dram_tensor("b_qkv", [P, 3], F32, kind="ExternalInput")
    w_p = nc.dram_tensor("w_p", [CD, D], F32R, kind="ExternalInput")
    trid = nc.dram_tensor("trid", [P, P], BF16, kind="ExternalInput")
    iden_b = nc.dram_tensor("iden_b", [P, P], BF16, kind="ExternalInput")
    iden_f = nc.dram_tensor("iden_f", [P, P], F32, kind="ExternalInput")
    vones = nc.dram_tensor("vones", [P, NSC, HPC], BF16, kind="ExternalInput")
    ones_r = nc.dram_tensor("ones_r", [1, P], F32R, kind="ExternalInput")
    out = nc.dram_tensor("out", [S, D], BF16, kind="ExternalOutput")

    with (
        tile.TileContext(nc) as tc,
        ExitStack() as ctx,
        nc.allow_low_precision(reason="bf16 matmul pipeline"),
    ):
        const = ctx.enter_context(tc.tile_pool(name="const", bufs=1))
        work = ctx.enter_context(tc.tile_pool(name="work", bufs=2))
        pp = ctx.enter_context(tc.tile_pool(name="pp", bufs=1, space="PSUM"))

        def psA(name):  # shared 2-bank psum ring
            return pp.tile([P, 2, 512], F32, tag="A", bufs=3, name=name)

        # ---- loads: small consts on gpsimd, first hidden half + weights on sync
        identb = const.tile([P, P], BF16, tag="identb", name="identb")
        nc.gpsimd.dma_start(out=identb, in_=iden_b.ap())
        identf = const.tile([P, P], F32, tag="identf", name="identf")
        nc.gpsimd.dma_start(out=identf, in_=iden_f.ap())
        hs_n = [
            const.tile([P, KO, 512], BF16, tag=f"hs{n}", name=f"hs{n}")
            for n in range(NQC)
        ]
        nc.sync.dma_start(out=hs_n[0][:, 0:4, :], in_=hsTd.ap()[:, 0, 0:4, :])
        wqkv_sb = const.tile([P, KO, 3 * P], BF16, tag="wq", name="wqkv_sb")
        nc.sync.dma_start(out=wqkv_sb, in_=wqd.ap())
        nc.sync.dma_start(out=hs_n[0][:, 4:8, :], in_=hsTd.ap()[:, 0, 4:8, :])

        onesr_sb = const.tile([1, P], F32R, tag="onesr", name="onesr_sb")
        nc.gpsimd.dma_start(out=onesr_sb, in_=ones_r.ap())
        bqkv_sb = const.tile([P, 3], F32, tag="bqkv", name="bqkv_sb")
        nc.gpsimd.dma_start(out=bqkv_sb, in_=b_qkv.ap())
        tri_sb = const.tile([P, P], BF16, tag="tri", name="tri_sb")
        nc.gpsimd.dma_start(out=tri_sb, in_=trid.ap())
        v2_sb = const.tile([P, NSC, HPC, HS + 1], BF16, tag="v2", name="v2_sb")
        nc.gpsimd.dma_start(out=v2_sb[:, :, :, HS], in_=vones.ap())

        nc.sync.dma_start(out=hs_n[1], in_=hsTd.ap()[:, 1, :, :])
        nc.gpsimd.dma_start(out=hs_n[2], in_=hsTd.ap()[:, 2, :, :])

        wp_sb = const.tile([P, D], F32R, tag="wp", name="wp_sb")
        qkT_sb = const.tile([P, 2, S], BF16, tag="qkT", name="qkT_sb")
        vT_sb = const.tile([P, S], BF16, tag="vT", name="vT_sb")
        u2_sb = [
            const.tile([P, 512], F32R, tag=f"u2_{qc}", name=f"u2_{qc}")
            for qc in range(NQC)
        ]
        u2n_sb = [
            const.tile([P, 512], F32R, tag=f"u2n_{qc}", name=f"u2n_{qc}")
            for qc in range(NQC)
        ]
        den2_sb = [
            const.tile([1, HPC, 512], F32, tag=f"den_{qc}", name=f"den_{qc}")
            for qc in range(NQC)
        ]

        # ---- PE warm-up paced by the first half-chunk
        ps_w = psA("ps_w")
        for o in range(4):
            for rep in range(3):
                nc.tensor.matmul(
                    ps_w[:, 0, :],
                    lhsT=identb,
                    rhs=hs_n[0][:, o, :],
                    start=True,
                    stop=True,
                )

        # ---- phase 1: q, k, v projections ([dim, s] layout) + v transposes
        def emit_p1_m(n, m):
            ps_m = psA("ps_qkv")[:, 0, :]
            for o in range(KO):
                nc.tensor.matmul(
                    ps_m,
                    lhsT=wqkv_sb[:, o, m * P : (m + 1) * P],
                    rhs=hs_n[n][:, o, :],
                    start=(o == 0),
                    stop=(o == KO - 1),
                )
            cols = slice(n * 512, (n + 1) * 512)
            dst = qkT_sb[:, m, cols] if m < 2 else vT_sb[:, cols]
            nc.vector.tensor_scalar_add(
                out=dst, in0=ps_m, scalar1=bqkv_sb[:, m : m + 1]
            )

        def emit_p1_t(sc):
            ps_t = pp.tile([P, P], BF16, tag="A", bufs=3, name="ps_t")
            nc.tensor.transpose(ps_t, vT_sb[:, sc * P : (sc + 1) * P], identb)
            nc.vector.tensor_copy(
                out=v2_sb[:, sc, :, 0:HS],
                in_=ps_t.rearrange("p (a b) -> p a b", a=HPC),
            )

        for m in range(3):
            emit_p1_m(0, m)
        for sc in range(4):
            emit_p1_t(sc)

        # ---- norm: den rows -> PE transpose -> wide DVE recip -> transpose
        # back -> K=1 broadcast -> one multiply per head
        def emit_norm(qc):
            dcols = pp.tile([P, 4, HPC], F32, tag="A", bufs=3, name="dcols")
            for j in range(4):
                for h in range(HPC):
                    # den row -> column: out[p,0] = den[128j+p] via K=1 matmul
                    nc.tensor.matmul(
                        dcols[:, j, h : h + 1],
                        lhsT=den2_sb[qc][:, h, j * P : (j + 1) * P],
                        rhs=identf[0:1, 0:1],
                        start=True,
                        stop=True,
                    )
            rcols = work.tile([P, 4, HPC], F32, tag="rc", bufs=2, name="rcols")
            nc.vector.reciprocal(out=rcols, in_=dcols)
            rrows = []
            for h in range(HPC):
                rrow_ps = pp.tile([1, 512], F32, tag="A", bufs=3, name="rrow_ps")
                for j in range(4):
                    # column -> row: out[0,c] = rcols[c] via M=1 matmul
                    nc.tensor.matmul(
                        rrow_ps[:, j * P : (j + 1) * P],
                        lhsT=rcols[:, j, h : h + 1],
                        rhs=identf,
                        start=True,
                        stop=True,
                    )
                rrow_sb = work.tile(
                    [1, 512], F32R, tag=f"rr{h}", bufs=2, name="rrow_sb"
                )
                nc.vector.tensor_copy(out=rrow_sb, in_=rrow_ps)
                rrows.append(rrow_sb)
            rb_ps = psA("ps_rb")
            for h in range(HPC):
                nc.tensor.matmul(
                    rb_ps[0:HS, h, :],
                    lhsT=onesr_sb[:, 0:HS],
                    rhs=rrows[h],
                    start=True,
                    stop=True,
                )
            for h in range(HPC):
                nc.vector.tensor_mul(
                    out=u2n_sb[qc][h * HS : (h + 1) * HS, :],
                    in0=u2_sb[qc][h * HS : (h + 1) * HS, :],
                    in1=rb_ps[0:HS, h, :],
                )

        # ---- phase 3: projection chunk + paired bf16 output DMA
        out_t = {}

        def emit_p3(sc):
            qc = sc // 4
            f = sc % 4
            slot = psA("ps_p3")
            for dc in range(2):
                nc.tensor.matmul(
                    slot[:, dc, :],
                    lhsT=u2n_sb[qc][:, f * P : (f + 1) * P],
                    rhs=wp_sb[:, dc * 512 : (dc + 1) * 512],
                    start=True,
                    stop=True,
                )
            pair = sc // 2
            if sc % 2 == 0:
                out_t[pair] = work.tile(
                    [P, 2, 2, 512], BF16, tag="out", bufs=2, name="out_t"
                )
            nc.vector.tensor_copy(out=out_t[pair][:, sc % 2], in_=slot)
            if sc % 2 == 1:
                eng = nc.sync if pair % 2 == 0 else nc.gpsimd
                # dram rows r = 128*j + p -> dims (p, j, c) to match src order
                dst = out.ap()[
                    (pair * 2) * P : (pair * 2 + 2) * P, :
                ].rearrange("(j p) c -> p j c", j=2)
                eng.dma_start(
                    out=dst, in_=out_t[pair].rearrange("p a b c -> p a (b c)")
                )

        # ---- phase 2: causal attention with deadline-tagged PE fillers
        fillers = deque()

        def drain(k):
            for _ in range(min(k, len(fillers))):
                fillers.popleft()[1]()

        def drain_due(qc):
            rest = deque()
            while fillers:
                d, fn = fillers.popleft()
                if d <= qc:
                    fn()
                else:
                    rest.append((d, fn))
            fillers.extend(rest)

        for qc in range(NQC):
            if qc + 1 < NQC:
                n = qc + 1
                for m in range(3):
                    fillers.append((n, lambda n=n, m=m: emit_p1_m(n, m)))
                for sc in range(4 * n, 4 * n + 4):
                    fillers.append((n, lambda sc=sc: emit_p1_t(sc)))
                if n + 1 < NQC:
                    fillers.append(
                        (n + 1, lambda n=n: nc.gpsimd.dma_start(
                            out=hs_n[n + 1], in_=hsTd.ap()[:, n + 1, :, :]
                        ))
                    )
                if n == 1:
                    fillers.append(
                        (NQC, lambda: nc.sync.dma_start(out=wp_sb, in_=w_p.ap()))
                    )
            drain_due(qc)

            ps_o = [
                pp.tile([P, 512], F32, tag="O", bufs=2, name=f"ps_o{h}")
                for h in range(HPC)
            ]
            nkb = 4 * (qc + 1)
            ngrp = nkb // 2

            def emit_pv(pend, qc=qc, nkb=nkb, ps_o=ps_o):
                pes, kbs = pend
                for h in range(HPC):
                    for j, kb in enumerate(kbs):
                        c0 = 128 * (kb - 4 * qc) if kb >= 4 * qc else 0
                        nc.tensor.matmul(
                            ps_o[h][0 : HS + 1, c0:512],
                            lhsT=v2_sb[:, kb, h, :],
                            rhs=pes[h][:, j, c0:512],
                            start=(kb == 0),
                            stop=(kb == nkb - 1),
                        )

            pending = None
            for g in range(ngrp):
                kbs = [2 * g, 2 * g + 1]
                f0 = 256 if g == ngrp - 1 else 0
                ps_att = [psA(f"ps_att{h}") for h in range(HPC)]
                for j, kb in enumerate(kbs):
                    for h in range(HPC):
                        nc.tensor.matmul(
                            ps_att[h][:, j, f0:512],
                            lhsT=qkT_sb[
                                h * HS : (h + 1) * HS, 1, kb * P : (kb + 1) * P
                            ],
                            rhs=qkT_sb[
                                h * HS : (h + 1) * HS,
                                0,
                                qc * 512 + f0 : (qc + 1) * 512,
                            ],
                            start=True,
                            stop=True,
                        )
                if pending is not None:
                    emit_pv(pending)
                if g == 0 and qc >= 1:
                    emit_norm(qc - 1)
                    for sc in range(4 * (qc - 1), 4 * qc):
                        fillers.append((NQC, lambda sc=sc: emit_p3(sc)))
                drain(1)
                pes = []
                for h in range(HPC):
                    p_exp = work.tile(
                        [P, 2, 512], BF16, tag=f"pe{h}", bufs=4, name="p_exp"
                    )
                    nc.scalar.activation(
                        out=p_exp[:, :, f0:512],
                        in_=ps_att[h][:, :, f0:512],
                        func=mybir.ActivationFunctionType.Exp,
                        scale=SCALE,
                    )
                    for j, kb in enumerate(kbs):
                        jj = kb - 4 * qc
                        if jj >= 0:  # mask only the exact diagonal sub-block
                            c0 = 128 * jj
                            nc.vector.tensor_mul(
                                out=p_exp[:, j, c0 : c0 + P],
                                in0=p_exp[:, j, c0 : c0 + P],
                                in1=tri_sb,
                            )
                    pes.append(p_exp)
                pending = (pes, kbs)
            emit_pv(pending)

            for h in range(HPC):
                nc.vector.tensor_copy(
                    out=u2_sb[qc][h * HS : (h + 1) * HS, :], in_=ps_o[h][0:HS, :]
                )
                nc.vector.tensor_copy(
                    out=den2_sb[qc][:, h, :], in_=ps_o[h][HS : HS + 1, :]
                )

        emit_norm(NQC - 1)
        for sc in range(4 * (NQC - 1), 4 * NQC):
            fillers.append((NQC, lambda sc=sc: emit_p3(sc)))
        drain(len(fillers))

    nc.compile()
    return nc


_NC = None


def _get_nc():
    global _NC
    if _NC is None:
        _NC = _build()
    return _NC


def prepare_inputs(hidden_states, W_attn, b_attn, W_proj, b_proj):
    hs = np.asarray(hidden_states, dtype=np.float32)
    Wa = np.asarray(W_attn, dtype=np.float32)
    ba = np.asarray(b_attn, dtype=np.float32)
    Wp = np.asarray(W_proj, dtype=np.float32)

    hsT = np.ascontiguousarray(hs.T).astype(NP_BF16)  # [D, S]
    hsTd = np.ascontiguousarray(
        hsT.reshape(KO, P, NQC, 512).transpose(1, 2, 0, 3)
    )

    pcol = np.arange(P)[:, None]
    ccol = np.arange(P)[None, :]
    tri = (pcol <= ccol).astype(NP_BF16)

    in_maps = []
    for c in range(NCORES):
        q0 = c * CD
        wq = Wa[:, q0 : q0 + CD]
        wk = Wa[:, D + q0 : D + q0 + CD]
        wv = Wa[:, 2 * D + q0 : 2 * D + q0 + CD]
        wcat = np.concatenate([wq, wk, wv], axis=1).astype(NP_BF16)  # [D, 384]
        wqd = np.ascontiguousarray(wcat.reshape(KO, P, 3 * P).transpose(1, 0, 2))
        bq = ba[q0 : q0 + CD]
        bk = ba[D + q0 : D + q0 + CD]
        bv = ba[2 * D + q0 : 2 * D + q0 + CD]
        in_maps.append(
            {
                "hsTd": hsTd,
                "wqd": wqd,
                "b_qkv": np.ascontiguousarray(
                    np.stack([bq, bk, bv], axis=1)
                ).astype(np.float32),
                "w_p": np.ascontiguousarray(Wp[q0 : q0 + CD, :], dtype=np.float32),
                "trid": tri,
                "iden_b": np.eye(P).astype(NP_BF16),
                "iden_f": np.eye(P).astype(np.float32),
                "ones_r": np.ones((1, P), dtype=np.float32),
                "vones": np.ones((P, NSC, HPC)).astype(NP_BF16),
            }
        )
    return in_maps


def run(inputs, trace=False):
    """Build+run the sharded kernel. Returns (full_output, BassKernelResults)."""
    in_maps = prepare_inputs(**inputs)
    nc = _get_nc()
    res = run_bass_kernel_spmd(
        nc, in_maps, core_ids=list(range(NCORES)), trace=trace
    )
    acc = np.zeros((S, D), dtype=np.float32)
    for c in range(NCORES):
        acc += res.results[c]["out"].astype(np.float32)
    acc += np.asarray(inputs["b_proj"], dtype=np.float32)
    return acc, res


def kernel(**inputs):
    out, _ = run(inputs, trace=False)
    return out


# revision 17
# speedup vs baseline: 1.3616x; 1.1390x over previous
"""Causal self-attention (S=2048, D=1024, H=16) on 8 Trainium2 NeuronCores.

Sharding: tensor-parallel over heads; core c owns heads 2c, 2c+1.

v2.1 pipeline (bf16 matmuls, restructured schedule):
  - Inputs stream as 4 chunk-contiguous 1MB descriptors (first chunk split in
    half so QKV work starts ~2us in); weights in one descriptor each.
  - Phase 1 (QKV projections into contraction layout + PE transposes of V into
    natural layout) is emitted as "filler" work interleaved between attention
    groups of the previous query chunk, keeping the in-order PE queue dense
    while ScalarE runs exp.
  - Causal masking at 128x128 sub-block granularity: only the exact diagonal
    sub-block gets a DVE mask multiply; fully-hidden regions are skipped by
    column-slicing the PV matmuls.
  - Softmax denominators: PE-transpose den rows into columns, one wide DVE
    reciprocal on [128, 4, 2] (avoids the pathological single-lane [1,512]
    reciprocal and the Ln/Exp act-table thrash), PE-transpose back, broadcast
    via K=1 matmul. Norm for chunk qc is emitted after the first group of
    qc+1 so the PE never stalls on it.
  - Projection chunks run right after their chunk's norm; output is written as
    bf16 in paired 256-row descriptors (host sums partials in f32 + b_proj).
"""

import math
from collections import deque
from contextlib import ExitStack

import numpy as np

import concourse.bacc as bacc
import concourse.mybir as mybir
import concourse.tile as tile
from concourse.bass_utils import run_bass_kernel_spmd

S, D, H = 2048, 1024, 16
HS = D // H  # 64 head size
P = 128
NCORES = 8
HPC = H // NCORES  # 2 heads per core
CD = HPC * HS  # 128 per-core head dims
KO = D // P  # 8 contraction tiles
NQC = S // 512  # 4 query chunks
NSC = S // P  # 16 sequence chunks of 128
SCALE = 1.0 / math.sqrt(S)

F32 = mybir.dt.float32
F32R = mybir.dt.float32r
BF16 = mybir.dt.bfloat16

import ml_dtypes

NP_BF16 = ml_dtypes.bfloat16


def _build():
    nc = bacc.Bacc(
        "TRN2", target_bir_lowering=False, debug=False, num_devices=NCORES
    )

    hsTd = nc.dram_tensor("hsTd", [P, NQC, KO, 512], BF16, kind="ExternalInput")
    wqd = nc.dram_tensor("wqd", [P, KO, 3 * P], BF16, kind="ExternalInput")
    b_qkv = nc.dram_tensor("b_qkv", [P, 3], F32, kind="ExternalInput")
    w_p = nc.dram_tensor("w_p", [CD, D], F32R, kind="ExternalInput")
    trid = nc.dram_tensor("trid", [P, P], BF16, kind="ExternalInput")
    iden_b = nc.dram_tensor("iden_b", [P, P], BF16, kind="ExternalInput")
    vones = nc.dram_tensor("vones", [P, NSC, HPC], BF16, kind="ExternalInput")
    ones_r = nc.dram_tensor("ones_r", [1, P], F32R, kind="ExternalInput")
    out = nc.dram_tensor("out", [S, D], BF16, kind="ExternalOutput")

    with (
        tile.TileContext(nc) as tc,
        ExitStack() as ctx,
        nc.allow_low_precision(reason="bf16 matmul pipeline"),
    ):
        const = ctx.enter_context(tc.tile_pool(name="const", bufs=1))
        work = ctx.enter_context(tc.tile_pool(name="work", bufs=2))
        pp = ctx.enter_context(tc.tile_pool(name="pp", bufs=1, space="PSUM"))

        def psA(name):  # shared 2-bank psum ring
            return pp.tile([P, 2, 512], F32, tag="A", bufs=3, name=name)

        # ---- loads: small consts on gpsimd, first hidden half + weights on sync
        identb = const.tile([P, P], BF16, tag="identb", name="identb")
        nc.sync.dma_start(out=identb, in_=iden_b.ap())
        hs_n = [
            const.tile([P, KO, 512], BF16, tag=f"hs{n}", name=f"hs{n}")
            for n in range(NQC)
        ]
        nc.sync.dma_start(out=hs_n[0][:, 0:4, :], in_=hsTd.ap()[:, 0, 0:4, :])
        wqkv_sb = const.tile([P, KO, 3 * P], BF16, tag="wq", name="wqkv_sb")
        nc.gpsimd.dma_start(out=wqkv_sb, in_=wqd.ap())
        nc.sync.dma_start(out=hs_n[0][:, 4:8, :], in_=hsTd.ap()[:, 0, 4:8, :])

        onesr_sb = const.tile([1, P], F32R, tag="onesr", name="onesr_sb")
        nc.gpsimd.dma_start(out=onesr_sb, in_=ones_r.ap())
        bqkv_sb = const.tile([P, 3], F32, tag="bqkv", name="bqkv_sb")
        nc.gpsimd.dma_start(out=bqkv_sb, in_=b_qkv.ap())
        tri_sb = const.tile([P, P], BF16, tag="tri", name="tri_sb")
        nc.gpsimd.dma_start(out=tri_sb, in_=trid.ap())
        v2_sb = const.tile([P, NSC, HPC, HS + 1], BF16, tag="v2", name="v2_sb")
        nc.gpsimd.dma_start(out=v2_sb[:, :, :, HS], in_=vones.ap())

        nc.sync.dma_start(out=hs_n[1], in_=hsTd.ap()[:, 1, :, :])
        nc.gpsimd.dma_start(out=hs_n[2], in_=hsTd.ap()[:, 2, :, :])

        wp_sb = const.tile([P, D], F32R, tag="wp", name="wp_sb")
        qkT_sb = const.tile([P, 2, S], BF16, tag="qkT", name="qkT_sb")
        vT_sb = const.tile([P, S], BF16, tag="vT", name="vT_sb")
        u2_sb = [
            const.tile([P, 512], F32R, tag=f"u2_{qc}", name=f"u2_{qc}")
            for qc in range(NQC)
        ]
        u2n_sb = [
            const.tile([P, 512], F32R, tag=f"u2n_{qc}", name=f"u2n_{qc}")
            for qc in range(NQC)
        ]
        den2_sb = [
            const.tile([1, HPC, 512], F32, tag=f"den_{qc}", name=f"den_{qc}")
            for qc in range(NQC)
        ]

        # preload the ln+exp activation table set once (no thrash later)
        from concourse.hw_specs import get_activation_tables

        _tables = list(get_activation_tables(nc.m.arch).keys())
        nc.scalar.add_instruction(
            mybir.InstLoadActFuncSet(
                name=nc.get_next_instruction_name(),
                ins=[],
                outs=[],
                act_func_set_id=_tables.index("natural_log_exp_and_others"),
            )
        )

        # ---- PE warm-up paced by the first half-chunk
        ps_w = psA("ps_w")
        for o in range(4):
            for rep in range(3):
                nc.tensor.matmul(
                    ps_w[:, 0, :],
                    lhsT=identb,
                    rhs=hs_n[0][:, o, :],
                    start=True,
                    stop=True,
                )

        # ---- phase 1: q, k, v projections ([dim, s] layout) + v transposes
        def emit_p1_m(n, m):
            ps_m = psA("ps_qkv")[:, 0, :]
            for o in range(KO):
                nc.tensor.matmul(
                    ps_m,
                    lhsT=wqkv_sb[:, o, m * P : (m + 1) * P],
                    rhs=hs_n[n][:, o, :],
                    start=(o == 0),
                    stop=(o == KO - 1),
                )
            cols = slice(n * 512, (n + 1) * 512)
            dst = qkT_sb[:, m, cols] if m < 2 else vT_sb[:, cols]
            nc.vector.tensor_scalar_add(
                out=dst, in0=ps_m, scalar1=bqkv_sb[:, m : m + 1]
            )

        def emit_p1_t(sc):
            ps_t = pp.tile([P, P], BF16, tag="A", bufs=3, name="ps_t")
            nc.tensor.transpose(ps_t, vT_sb[:, sc * P : (sc + 1) * P], identb)
            nc.vector.tensor_copy(
                out=v2_sb[:, sc, :, 0:HS],
                in_=ps_t.rearrange("p (a b) -> p a b", a=HPC),
            )

        for m in range(3):
            emit_p1_m(0, m)
        for sc in range(4):
            emit_p1_t(sc)

        # ---- norm: Ln/Exp reciprocal rows on ScalarE (the combined
        # natural_log_exp_and_others table set is preloaded once, so there is
        # no act-table thrash), K=1 broadcast, one multiply per head. Norm for
        # chunk qc is emitted one group into qc+1 so the PE queue never waits.
        def emit_norm(qc):
            rrows = []
            for h in range(HPC):
                lg = work.tile([1, 512], F32, tag=f"lg{h}", bufs=2, name="lg")
                nc.scalar.activation(
                    out=lg,
                    in_=den2_sb[qc][:, h, :],
                    func=mybir.ActivationFunctionType.Ln,
                )
                rrow = work.tile([1, 512], F32R, tag=f"rr{h}", bufs=2, name="rrow")
                nc.scalar.activation(
                    out=rrow,
                    in_=lg,
                    func=mybir.ActivationFunctionType.Exp,
                    scale=-1.0,
                )
                rrows.append(rrow)
            rb_ps = psA("ps_rb")
            for h in range(HPC):
                nc.tensor.matmul(
                    rb_ps[0:HS, h, :],
                    lhsT=onesr_sb[:, 0:HS],
                    rhs=rrows[h],
                    start=True,
                    stop=True,
                )
            for h in range(HPC):
                nc.vector.tensor_mul(
                    out=u2n_sb[qc][h * HS : (h + 1) * HS, :],
                    in0=u2_sb[qc][h * HS : (h + 1) * HS, :],
                    in1=rb_ps[0:HS, h, :],
                )

        # ---- phase 3: projection chunk + paired bf16 output DMA
        out_t = {}

        def emit_p3(sc):
            qc = sc // 4
            f = sc % 4
            slot = psA("ps_p3")
            for dc in range(2):
                nc.tensor.matmul(
                    slot[:, dc, :],
                    lhsT=u2n_sb[qc][:, f * P : (f + 1) * P],
                    rhs=wp_sb[:, dc * 512 : (dc + 1) * 512],
                    start=True,
                    stop=True,
                )
            pair = sc // 2
            if sc % 2 == 0:
                out_t[pair] = work.tile(
                    [P, 2, 2, 512], BF16, tag="out", bufs=2, name="out_t"
                )
            nc.vector.tensor_copy(out=out_t[pair][:, sc % 2], in_=slot)
            if sc % 2 == 1:
                eng = nc.sync if pair % 2 == 0 else nc.gpsimd
                # dram rows r = 128*j + p -> dims (p, j, c) to match src order
                dst = out.ap()[
                    (pair * 2) * P : (pair * 2 + 2) * P, :
                ].rearrange("(j p) c -> p j c", j=2)
                eng.dma_start(
                    out=dst, in_=out_t[pair].rearrange("p a b c -> p a (b c)")
                )

        # ---- phase 2: causal attention with deadline-tagged PE fillers
        fillers = deque()

        def drain(k):
            for _ in range(min(k, len(fillers))):
                fillers.popleft()[1]()

        def drain_due(qc):
            rest = deque()
            while fillers:
                d, fn = fillers.popleft()
                if d <= qc:
                    fn()
                else:
                    rest.append((d, fn))
            fillers.extend(rest)

        for qc in range(NQC):
            if qc + 1 < NQC:
                n = qc + 1
                for m in range(3):
                    fillers.append((n, lambda n=n, m=m: emit_p1_m(n, m)))
                for sc in range(4 * n, 4 * n + 4):
                    fillers.append((n, lambda sc=sc: emit_p1_t(sc)))
                if n + 1 < NQC:
                    fillers.append(
                        (n + 1, lambda n=n: nc.gpsimd.dma_start(
                            out=hs_n[n + 1], in_=hsTd.ap()[:, n + 1, :, :]
                        ))
                    )
                if n == 1:
                    fillers.append(
                        (NQC, lambda: nc.sync.dma_start(out=wp_sb, in_=w_p.ap()))
                    )
            drain_due(qc)

            ps_o = [
                pp.tile([P, 512], F32, tag="O", bufs=2, name=f"ps_o{h}")
                for h in range(HPC)
            ]
            nkb = 4 * (qc + 1)
            ngrp = nkb // 2

            def emit_pv(pend, qc=qc, nkb=nkb, ps_o=ps_o):
                pes, kbs = pend
                for h in range(HPC):
                    for j, kb in enumerate(kbs):
                        c0 = 128 * (kb - 4 * qc) if kb >= 4 * qc else 0
                        nc.tensor.matmul(
                            ps_o[h][0 : HS + 1, c0:512],
                            lhsT=v2_sb[:, kb, h, :],
                            rhs=pes[h][:, j, c0:512],
                            start=(kb == 0),
                            stop=(kb == nkb - 1),
                        )

            pending = None
            for g in range(ngrp):
                kbs = [2 * g, 2 * g + 1]
                f0 = 256 if g == ngrp - 1 else 0
                ps_att = [psA(f"ps_att{h}") for h in range(HPC)]
                for j, kb in enumerate(kbs):
                    for h in range(HPC):
                        nc.tensor.matmul(
                            ps_att[h][:, j, f0:512],
                            lhsT=qkT_sb[
                                h * HS : (h + 1) * HS, 1, kb * P : (kb + 1) * P
                            ],
                            rhs=qkT_sb[
                                h * HS : (h + 1) * HS,
                                0,
                                qc * 512 + f0 : (qc + 1) * 512,
                            ],
                            start=True,
                            stop=True,
                        )
                if pending is not None:
                    emit_pv(pending)
                if g == 1 and qc >= 1:
                    emit_norm(qc - 1)
                    for sc in range(4 * (qc - 1), 4 * qc):
                        fillers.append((NQC, lambda sc=sc: emit_p3(sc)))
                drain(1)
                pes = []
                for h in range(HPC):
                    p_exp = work.tile(
                        [P, 2, 512], BF16, tag=f"pe{h}", bufs=4, name="p_exp"
                    )
                    nc.scalar.activation(
                        out=p_exp[:, :, f0:512],
                        in_=ps_att[h][:, :, f0:512],
                        func=mybir.ActivationFunctionType.Exp,
                        scale=SCALE,
                    )
                    for j, kb in enumerate(kbs):
                        jj = kb - 4 * qc
                        if jj >= 0:  # mask only the exact diagonal sub-block
                            c0 = 128 * jj
                            nc.vector.tensor_mul(
                                out=p_exp[:, j, c0 : c0 + P],
                                in0=p_exp[:, j, c0 : c0 + P],
                                in1=tri_sb,
                            )
                    pes.append(p_exp)
                pending = (pes, kbs)
            emit_pv(pending)

            for h in range(HPC):
                nc.vector.tensor_copy(
                    out=u2_sb[qc][h * HS : (h + 1) * HS, :], in_=ps_o[h][0:HS, :]
                )
                nc.vector.tensor_copy(
                    out=den2_sb[qc][:, h, :], in_=ps_o[h][HS : HS + 1, :]
                )

        emit_norm(NQC - 1)
        for sc in range(4 * (NQC - 1), 4 * NQC):
            fillers.append((NQC, lambda sc=sc: emit_p3(sc)))
        drain(len(fillers))

    nc.compile()
    return nc


_NC = None


def _get_nc():
    global _NC
    if _NC is None:
        _NC = _build()
    return _NC


def prepare_inputs(hidden_states, W_attn, b_attn, W_proj, b_proj):
    hs = np.asarray(hidden_states, dtype=np.float32)
    Wa = np.asarray(W_attn, dtype=np.float32)
    ba = np.asarray(b_attn, dtype=np.float32)
    Wp = np.asarray(W_proj, dtype=np.float32)

    hsT = np.ascontiguousarray(hs.T).astype(NP_BF16)  # [D, S]
    hsTd = np.ascontiguousarray(
        hsT.reshape(KO, P, NQC, 512).transpose(1, 2, 0, 3)
    )

    pcol = np.arange(P)[:, None]
    ccol = np.arange(P)[None, :]
    tri = (pcol <= ccol).astype(NP_BF16)

    in_maps = []
    for c in range(NCORES):
        q0 = c * CD
        wq = Wa[:, q0 : q0 + CD]
        wk = Wa[:, D + q0 : D + q0 + CD]
        wv = Wa[:, 2 * D + q0 : 2 * D + q0 + CD]
        wcat = np.concatenate([wq, wk, wv], axis=1).astype(NP_BF16)  # [D, 384]
        wqd = np.ascontiguousarray(wcat.reshape(KO, P, 3 * P).transpose(1, 0, 2))
        bq = ba[q0 : q0 + CD]
        bk = ba[D + q0 : D + q0 + CD]
        bv = ba[2 * D + q0 : 2 * D + q0 + CD]
        in_maps.append(
            {
                "hsTd": hsTd,
                "wqd": wqd,
                "b_qkv": np.ascontiguousarray(
                    np.stack([bq, bk, bv], axis=1)
                ).astype(np.float32),
                "w_p": np.ascontiguousarray(Wp[q0 : q0 + CD, :], dtype=np.float32),
                "trid": tri,
                "iden_b": np.eye(P).astype(NP_BF16),
                "ones_r": np.ones((1, P), dtype=np.float32),
                "vones": np.ones((P, NSC, HPC)).astype(NP_BF16),
            }
        )
    return in_maps


def run(inputs, trace=False):
    """Build+run the sharded kernel. Returns (full_output, BassKernelResults)."""
    in_maps = prepare_inputs(**inputs)
    nc = _get_nc()
    res = run_bass_kernel_spmd(
        nc, in_maps, core_ids=list(range(NCORES)), trace=trace
    )
    acc = np.zeros((S, D), dtype=np.float32)
    for c in range(NCORES):
        acc += res.results[c]["out"].astype(np.float32)
    acc += np.asarray(inputs["b_proj"], dtype=np.float32)
    return acc, res


def kernel(**inputs):
    out, _ = run(inputs, trace=False)
    return out
